# revision 4
# baseline (speedup 1.0000x reference)
"""BitLinear (1-bit packed weights) matmul kernel for 8 Trainium2 NeuronCores.

Computes out = x @ w.T, w[o,k] in {-1,+1} unpacked from bytes bp (MSB-first).

Strategy (tensor-parallel over out features, x replicated):
  - Each core owns OUT_F/8 = 1376 out features, padded to 1408 = 11 blocks
    of 128.
  - Contraction split by precision: k in [0, 2048) runs in bf16
    (negligible error), k in [2048, 4096) runs in fp8 e4m3 with
    perf_mode=DoubleRow (2 fp8 weights per PE cell -> 2x contraction per
    cycle). Weights ship as fp8 e4m3 (+-1 exact) even for the bf16-x half
    (mixed-dtype matmul runs at full rate), so no rowsum
    correction is needed; the only error is e4m3 rounding of half of x
    (~1.9e-2 norm rel err, under the 2e-2 gate).
  - o-stationary layout: stationary = weight block [128k, 128o], moving =
    x [128k, 512t]; psum [128o, 512t]. One weight load serves both token
    halves. Output DMAs are o-major ([1408, 1024] f32 per core, 4 KB per
    partition descriptor); host transposes at the end.
  - PSUM: groups of 4 o-blocks x 2 token-halves = 8 banks in flight.

Host-side prep is layout/sharding only: transpose + dtype-cast of x,
byte unpack + transpose of the weight matrix.
"""

from contextlib import ExitStack

import numpy as np
import ml_dtypes

import concourse.bass as bass
import concourse.mybir as mybir
import concourse.tile as tile
from concourse.bass_utils import run_bass_kernel_spmd


def _ensure_axon_hooks_module():
    """concourse's trace path imports antenv.axon_hooks unconditionally when
    BASS_TRACE is set; some images lack it. Provide a stub so tracing
    degrades gracefully instead of crashing."""
    try:
        import antenv.axon_hooks  # noqa: F401
    except ImportError:
        import sys
        import types

        import antenv

        mod = types.ModuleType("antenv.axon_hooks")
        mod._hook = None

        def set_axon_ntff_profile_hook(h, _mod=mod):
            _mod._hook = h

        def get_axon_ntff_profile_hook(_mod=mod):
            return _mod._hook

        mod.set_axon_ntff_profile_hook = set_axon_ntff_profile_hook
        mod.get_axon_ntff_profile_hook = get_axon_ntff_profile_hook
        sys.modules["antenv.axon_hooks"] = mod
        antenv.axon_hooks = mod


_ensure_axon_hooks_module()

TOKENS, IN_F, OUT_F = 1024, 4096, 11008
N_CORES = 8
OS = OUT_F // N_CORES        # 1376 out features per core
NOB = 11                     # o-blocks of 128 (padded)
OPAD = NOB * 128             # 1408
KS_BF = 16                   # bf16 k-subtiles (k 0..2047)
NPAIR = 8                    # fp8 DoubleRow pairs (k 2048..4095)
KF0 = KS_BF * 128            # fp8 k range start
OGS = [[0, 1, 2, 3], [4, 5, 6, 7], [10, 8, 9]]

FP8 = ml_dtypes.float8_e4m3
BF16 = ml_dtypes.bfloat16

_CACHE: dict = {}

_MAX_WAITS = 1  # walrus codegen rejects instructions with more sem waits
_HOIST_PE_WAITS = True  # move matmul waits onto NoOps so LDWEIGHTS pulls ahead


def _legalize_waits(nc) -> int:
    """Split instructions carrying >_MAX_WAITS sem waits into preceding
    same-engine NoOps (Tile's tail drain aggregates one wait per live
    semaphore, which walrus codegen rejects)."""
    n_split = 0
    for fn in nc.m.functions:
        for bb in fn.blocks:
            insts = list(bb.instructions)
            out = []
            for inst in insts:
                si = getattr(inst, "sync_info", None)
                waits = list(si.on_wait) if (si is not None and si.on_wait) else []
                hoist_all = (
                    _HOIST_PE_WAITS
                    and isinstance(inst, mybir.InstMatmult)
                    and waits
                )
                if len(waits) > _MAX_WAITS or hoist_all:
                    extra = waits if hoist_all else waits[:-_MAX_WAITS]
                    keep = [] if hoist_all else waits[-_MAX_WAITS:]
                    for i in range(0, len(extra), _MAX_WAITS):
                        chunk = extra[i:i + _MAX_WAITS]
                        out.append(mybir.InstNoOp(
                            name=f"{inst.name}_wsplit{i}",
                            engine=inst.engine,
                            ins=[],
                            outs=[],
                            sync_info=mybir.SyncInfo(on_wait=chunk, on_update=[]),
                        ))
                    si.on_wait = keep
                    n_split += 1
                out.append(inst)
            if len(out) != len(insts):
                bb.instructions[:] = out
    return n_split


def _build_module() -> bass.Bass:
    nc = bass.Bass(
        "TRN2",
        target_bir_lowering=False,
        debug=False,
        enable_asserts=False,
        num_devices=N_CORES,
    )
    xb_d = nc.dram_tensor(
        "xb", [128, KS_BF, TOKENS], mybir.dt.bfloat16, kind="ExternalInput"
    ).ap()
    xf_d = nc.dram_tensor(
        "xf", [128, 2 * NPAIR, TOKENS], mybir.dt.float8e4, kind="ExternalInput"
    ).ap()
    wb_d = nc.dram_tensor(
        "wb", [128, NOB, KS_BF * 128], mybir.dt.float8e4, kind="ExternalInput"
    ).ap()
    wf_d = nc.dram_tensor(
        "wf", [128, NOB, 2 * NPAIR * 128], mybir.dt.float8e4,
        kind="ExternalInput"
    ).ap()
    out_d = nc.dram_tensor(
        "out", [OPAD, TOKENS], mybir.dt.float32, kind="ExternalOutput"
    ).ap()

    DR = mybir.MatmulPerfMode.DoubleRow
    with ExitStack() as ctx:
        tc = ctx.enter_context(tile.TileContext(nc))
        sb = ctx.enter_context(tc.tile_pool(name="sb", bufs=1))
        wbp = ctx.enter_context(tc.tile_pool(name="wbp", bufs=7))
        wfp = ctx.enter_context(tc.tile_pool(name="wfp", bufs=7))
        opool = ctx.enter_context(tc.tile_pool(name="opool", bufs=4))
        ps = ctx.enter_context(tc.tile_pool(name="ps", bufs=1, space="PSUM"))

        # Weight streams on the ACT HWDGE ring, o-block granular,
        # in consumption order.
        wb_sb = {}
        wf_sb = {}

        def _wf_load(ob):
            wf_sb[ob] = wfp.tile(
                [128, 2 * NPAIR, 128], mybir.dt.float8e4, name=f"wf{ob}",
                tag="wf"
            )
            nc.scalar.dma_start(out=wf_sb[ob], in_=wf_d[:, ob, :])

        def _wb_load(ob):
            wb_sb[ob] = wbp.tile(
                [128, KS_BF, 128], mybir.dt.float8e4, name=f"wb{ob}", tag="wb"
            )
            nc.scalar.dma_start(out=wb_sb[ob], in_=wb_d[:, ob, :])

        xb_sb = sb.tile([128, KS_BF, TOKENS], mybir.dt.bfloat16, name="xb_sb")
        xf_sb = sb.tile(
            [128, 2 * NPAIR, TOKENS], mybir.dt.float8e4, name="xf_sb"
        )
        # SP ring carries the whole x stream in consumption order (xf for
        # og0's leading fp8 phase, then xb in fine 2-subtile chunks so any
        # shortfall stalls in sub-us pieces that don't re-throttle HAM).
        # ACT ring carries weights: og0's first, then the rest.
        for lo, hi in [(0, 2), (2, 6), (6, 10), (10, 14), (14, 16)]:
            nc.sync.dma_start(
                out=xf_sb[:, lo:hi, :], in_=xf_d[:, lo:hi, :]
            )
        # og0's fp8 weights in two waves: the j0-j1 quarters (64 KB each)
        # land first so the ob-major opening stretch starts ~4 us earlier.
        for ob in range(4):
            wf_sb[ob] = wfp.tile(
                [128, 2 * NPAIR, 128], mybir.dt.float8e4, name=f"wf{ob}",
                tag="wf"
            )
            nc.scalar.dma_start(
                out=wf_sb[ob][:, 0:4, :], in_=wf_d[:, ob, 0:512]
            )
        for ob in range(4):
            nc.scalar.dma_start(
                out=wf_sb[ob][:, 4:2 * NPAIR, :],
                in_=wf_d[:, ob, 512:2 * NPAIR * 128],
            )
        for ob in range(4):
            _wb_load(ob)
        for c in range(0, KS_BF, 2):
            nc.sync.dma_start(
                out=xb_sb[:, c:c + 2, :], in_=xb_d[:, c:c + 2, :]
            )
        for ob in range(4, NOB):
            _wb_load(ob)
            _wf_load(ob)

        # PE prewarm: dummy matmuls on memset tiles while the first weight
        # tile is in flight, so real MMs start at HAM 8/8 (2.4 GHz).
        warm_a = sb.tile([128, 128], mybir.dt.bfloat16, name="warm_a")
        nc.gpsimd.memset(warm_a, 0.0)
        warm_b = sb.tile([128, 512], mybir.dt.bfloat16, name="warm_b")
        nc.gpsimd.memset(warm_b, 0.0)
        warm_ps = ps.tile([128, 512], mybir.dt.float32, name="warm_ps", tag="ps0")
        for i in range(3):
            nc.tensor.matmul(
                warm_ps, lhsT=warm_a, rhs=warm_b,
                start=(i == 0), stop=(i == 2),
            )

        pst = {}

        def bf_phase(obs, starting):
            for ks in range(KS_BF):
                for ob in obs:
                    lhsT = wb_sb[ob][:, ks]
                    st = starting and ks == 0
                    sp = (not starting) and ks == KS_BF - 1
                    nc.tensor.matmul(
                        pst[ob][0], lhsT=lhsT, rhs=xb_sb[:, ks, 0:512],
                        start=st, stop=sp,
                    )
                    nc.tensor.matmul(
                        pst[ob][1], lhsT=lhsT, rhs=xb_sb[:, ks, 512:1024],
                        start=st, stop=sp,
                    )

        def f8_phase(obs, starting):
            for j in range(NPAIR):
                for ob in obs:
                    lhsT = wf_sb[ob][:, 2 * j:2 * j + 2, :]
                    st = starting and j == 0
                    sp = (not starting) and j == NPAIR - 1
                    nc.tensor.matmul(
                        pst[ob][0], lhsT=lhsT,
                        rhs=xf_sb[:, 2 * j:2 * j + 2, 0:512],
                        start=st, stop=sp, perf_mode=DR,
                    )
                    nc.tensor.matmul(
                        pst[ob][1], lhsT=lhsT,
                        rhs=xf_sb[:, 2 * j:2 * j + 2, 512:1024],
                        start=st, stop=sp, perf_mode=DR,
                    )

        def alloc_ps(ob):
            bank = (ob % 4) * 2
            pst[ob] = (
                ps.tile([128, 512], mybir.dt.float32, name=f"p{ob}a",
                        tag=f"ps{bank}"),
                ps.tile([128, 512], mybir.dt.float32, name=f"p{ob}b",
                        tag=f"ps{bank + 1}"),
            )

        def evict(ob):
            ot = opool.tile([128, TOKENS], mybir.dt.float32, name="ot",
                            tag="ot")
            if ob % 2 == 0:
                nc.scalar.activation(
                    ot[:, 0:512], pst[ob][0],
                    mybir.ActivationFunctionType.Identity,
                )
                nc.vector.tensor_copy(out=ot[:, 512:1024], in_=pst[ob][1])
            else:
                nc.vector.tensor_copy(out=ot[:, 0:512], in_=pst[ob][0])
                nc.scalar.activation(
                    ot[:, 512:1024], pst[ob][1],
                    mybir.ActivationFunctionType.Identity,
                )
            eng = nc.sync if ob % 2 == 0 else nc.scalar
            eng.dma_start(
                out=out_d[ob * 128:(ob + 1) * 128, 0:512],
                in_=ot[:, 0:512],
            )
            eng.dma_start(
                out=out_d[ob * 128:(ob + 1) * 128, 512:1024],
                in_=ot[:, 512:1024],
            )

        # og0: 4 o-blocks, fp8 planes first — x_bf16 is still streaming in.
        # The leading stretch runs ob-major over pairs 0-1 so the very first
        # matmuls need only wf0 + xf chunk 0; later weights/chunks arrive
        # under its cover.
        og0 = [0, 1, 2, 3]
        for ob in og0:
            alloc_ps(ob)
        for ob in og0:
            for j in range(2):
                lhsT = wf_sb[ob][:, 2 * j:2 * j + 2, :]
                for th in range(2):
                    nc.tensor.matmul(
                        pst[ob][th], lhsT=lhsT,
                        rhs=xf_sb[:, 2 * j:2 * j + 2, th * 512:th * 512 + 512],
                        start=(j == 0), stop=False, perf_mode=DR,
                    )
        for j in range(2, NPAIR):
            for ob in og0:
                lhsT = wf_sb[ob][:, 2 * j:2 * j + 2, :]
                for th in range(2):
                    nc.tensor.matmul(
                        pst[ob][th], lhsT=lhsT,
                        rhs=xf_sb[:, 2 * j:2 * j + 2, th * 512:th * 512 + 512],
                        start=False, stop=False, perf_mode=DR,
                    )
        bf_phase(og0, False)
        for ob in og0:
            evict(ob)

        # Remaining o-blocks run serialized: each finishes (and its output
        # DMA issues) while the next block's matmuls run, so evictions and
        # stores spread across the span instead of bunching at the end.
        for ob in range(4, NOB - 1):
            alloc_ps(ob)
            bf_phase([ob], True)
            f8_phase([ob], False)
            evict(ob)

        # Final o-block: token-half-major — half 0's accumulation closes
        # first and its eviction + store run under half 1's matmuls, so the
        # tail only carries half 1's eviction and DMA.
        ob = NOB - 1
        alloc_ps(ob)
        ot = opool.tile([128, TOKENS], mybir.dt.float32, name="ot", tag="ot")
        for th in range(2):
            pt = pst[ob][th]
            rhs_lo = th * 512
            for ks in range(KS_BF):
                nc.tensor.matmul(
                    pt, lhsT=wb_sb[ob][:, ks],
                    rhs=xb_sb[:, ks, rhs_lo:rhs_lo + 512],
                    start=(ks == 0), stop=False,
                )
            for j in range(NPAIR):
                nc.tensor.matmul(
                    pt, lhsT=wf_sb[ob][:, 2 * j:2 * j + 2, :],
                    rhs=xf_sb[:, 2 * j:2 * j + 2, rhs_lo:rhs_lo + 512],
                    start=False, stop=(j == NPAIR - 1), perf_mode=DR,
                )
            for q in range(2):
                lo = th * 512 + q * 256
                src = pt[:, q * 256:q * 256 + 256]
                if q == 0:
                    nc.scalar.activation(
                        ot[:, lo:lo + 256], src,
                        mybir.ActivationFunctionType.Identity,
                    )
                else:
                    nc.vector.tensor_copy(out=ot[:, lo:lo + 256], in_=src)
                eng = nc.sync if q == 0 else nc.scalar
                eng.dma_start(
                    out=out_d[ob * 128:(ob + 1) * 128, lo:lo + 256],
                    in_=ot[:, lo:lo + 256],
                )
    _legalize_waits(nc)
    return nc


def _prep_inputs(x: np.ndarray, bp: np.ndarray):
    x = np.ascontiguousarray(x, dtype=np.float32)
    # xb[ki, ks, t] = x[t, ks*128 + ki]   (bf16)
    xb = np.ascontiguousarray(
        x[:, :KF0].T.reshape(KS_BF, 128, TOKENS).transpose(1, 0, 2)
    ).astype(BF16)
    # xf[ki, s, t] = x[t, KF0 + s*128 + ki]   (e4m3)
    xf = np.ascontiguousarray(
        x[:, KF0:].T.reshape(2 * NPAIR, 128, TOKENS).transpose(1, 0, 2)
    ).astype(FP8)

    # weights: bytes -> {0,1} bits (MSB-first) -> +-1
    bytes_m = bp.reshape(OUT_F, IN_F // 8).astype(np.uint8)
    w01 = np.unpackbits(bytes_m, axis=1)            # [OUT_F, IN_F]
    w_pm = (w01.astype(np.int8) << 1) - 1           # {-1, +1}

    in_maps = []
    for c in range(N_CORES):
        wc = np.zeros((OPAD, IN_F), dtype=np.int8)
        wc[:OS] = w_pm[c * OS:(c + 1) * OS]
        # wb[ki, ob, ks, oj] = w[ob*128+oj, ks*128+ki]
        wb = np.ascontiguousarray(
            wc[:, :KF0].T                     # [2048, 1408]
            .reshape(KS_BF, 128, NOB, 128)    # [ks, ki, ob, oj]
            .transpose(1, 2, 0, 3)            # [ki, ob, ks, oj]
            .reshape(128, NOB, KS_BF * 128)
        ).astype(FP8)
        wf = np.ascontiguousarray(
            wc[:, KF0:].T
            .reshape(2 * NPAIR, 128, NOB, 128)
            .transpose(1, 2, 0, 3)
            .reshape(128, NOB, 2 * NPAIR * 128)
        ).astype(FP8)
        in_maps.append({"xb": xb, "xf": xf, "wb": wb, "wf": wf})
    return in_maps


def _run(x: np.ndarray, bp: np.ndarray, **spmd_kwargs):
    if "nc" not in _CACHE:
        _CACHE["nc"] = _build_module()
    nc = _CACHE["nc"]
    in_maps = _prep_inputs(x, bp)
    res = run_bass_kernel_spmd(
        nc, in_maps, core_ids=list(range(N_CORES)), **spmd_kwargs
    )
    out = np.ascontiguousarray(
        np.concatenate([r["out"][:OS] for r in res.results], axis=0).T
    )
    return out, res


def _host_reference(x: np.ndarray, bp: np.ndarray) -> np.ndarray:
    # Safety net for inputs outside the fast path's envelope.
    shifts = np.arange(7, -1, -1)
    bits = (bp.astype(np.int64)[:, None] >> shifts) & 1
    w = bits.reshape(OUT_F, IN_F).astype(np.float32) * 2 - 1
    return (x @ w.T).astype(np.float32)


def kernel(x: np.ndarray, bp: np.ndarray) -> np.ndarray:
    x = np.asarray(x, dtype=np.float32)
    bp = np.asarray(bp)
    # e4m3 covers |x| < 224 exactly in range; standard-normal inputs sit
    # near 5.2. Anything wilder goes through the host fallback.
    if (not np.isfinite(x).all()) or np.abs(x).max() >= 224.0 \
            or bp.min() < 0 or bp.max() > 255:
        return _host_reference(x, bp)
    out, _ = _run(x, bp)
    return out


# revision 5
# speedup vs baseline: 1.0155x; 1.0155x over previous
"""BitLinear (1-bit packed weights) matmul kernel for 8 Trainium2 NeuronCores.

Computes out = x @ w.T, w[o,k] in {-1,+1} unpacked from bytes bp (MSB-first).

Strategy (tensor-parallel over out features, x replicated):
  - Each core owns OUT_F/8 = 1376 out features, padded to 1408 = 11 blocks
    of 128.
  - Contraction split by precision: k in [0, 2048) runs in bf16
    (negligible error), k in [2048, 4096) runs in fp8 e4m3 with
    perf_mode=DoubleRow (2 fp8 weights per PE cell -> 2x contraction per
    cycle). Weights ship as fp8 e4m3 (+-1 exact) even for the bf16-x half
    (mixed-dtype matmul runs at full rate), so no rowsum
    correction is needed; the only error is e4m3 rounding of half of x
    (~1.9e-2 norm rel err, under the 2e-2 gate).
  - o-stationary layout: stationary = weight block [128k, 128o], moving =
    x [128k, 512t]; psum [128o, 512t]. One weight load serves both token
    halves. Output DMAs are o-major ([1408, 1024] f32 per core, 4 KB per
    partition descriptor); host transposes at the end.
  - PSUM: groups of 4 o-blocks x 2 token-halves = 8 banks in flight.

Host-side prep is layout/sharding only: transpose + dtype-cast of x,
byte unpack + transpose of the weight matrix.
"""

from contextlib import ExitStack

import numpy as np
import ml_dtypes

import concourse.bass as bass
import concourse.mybir as mybir
import concourse.tile as tile
from concourse.bass_utils import run_bass_kernel_spmd


def _ensure_axon_hooks_module():
    """concourse's trace path imports antenv.axon_hooks unconditionally when
    BASS_TRACE is set; some images lack it. Provide a stub so tracing
    degrades gracefully instead of crashing."""
    try:
        import antenv.axon_hooks  # noqa: F401
    except ImportError:
        import sys
        import types

        import antenv

        mod = types.ModuleType("antenv.axon_hooks")
        mod._hook = None

        def set_axon_ntff_profile_hook(h, _mod=mod):
            _mod._hook = h

        def get_axon_ntff_profile_hook(_mod=mod):
            return _mod._hook

        mod.set_axon_ntff_profile_hook = set_axon_ntff_profile_hook
        mod.get_axon_ntff_profile_hook = get_axon_ntff_profile_hook
        sys.modules["antenv.axon_hooks"] = mod
        antenv.axon_hooks = mod


_ensure_axon_hooks_module()

TOKENS, IN_F, OUT_F = 1024, 4096, 11008
N_CORES = 8
OS = OUT_F // N_CORES        # 1376 out features per core
NOB = 11                     # o-blocks of 128 (padded)
OPAD = NOB * 128             # 1408
KS_BF = 16                   # bf16 k-subtiles (k 0..2047)
NPAIR = 8                    # fp8 DoubleRow pairs (k 2048..4095)
KF0 = KS_BF * 128            # fp8 k range start
OGS = [[0, 1, 2, 3], [4, 5, 6, 7], [10, 8, 9]]

FP8 = ml_dtypes.float8_e4m3
BF16 = ml_dtypes.bfloat16

_CACHE: dict = {}

_MAX_WAITS = 1  # walrus codegen rejects instructions with more sem waits
_HOIST_PE_WAITS = True  # move matmul waits onto NoOps so LDWEIGHTS pulls ahead


def _legalize_waits(nc) -> int:
    """Split instructions carrying >_MAX_WAITS sem waits into preceding
    same-engine NoOps (Tile's tail drain aggregates one wait per live
    semaphore, which walrus codegen rejects)."""
    n_split = 0
    for fn in nc.m.functions:
        for bb in fn.blocks:
            insts = list(bb.instructions)
            out = []
            for inst in insts:
                si = getattr(inst, "sync_info", None)
                waits = list(si.on_wait) if (si is not None and si.on_wait) else []
                hoist_all = (
                    _HOIST_PE_WAITS
                    and isinstance(inst, mybir.InstMatmult)
                    and waits
                )
                if len(waits) > _MAX_WAITS or hoist_all:
                    extra = waits if hoist_all else waits[:-_MAX_WAITS]
                    keep = [] if hoist_all else waits[-_MAX_WAITS:]
                    for i in range(0, len(extra), _MAX_WAITS):
                        chunk = extra[i:i + _MAX_WAITS]
                        out.append(mybir.InstNoOp(
                            name=f"{inst.name}_wsplit{i}",
                            engine=inst.engine,
                            ins=[],
                            outs=[],
                            sync_info=mybir.SyncInfo(on_wait=chunk, on_update=[]),
                        ))
                    si.on_wait = keep
                    n_split += 1
                out.append(inst)
            if len(out) != len(insts):
                bb.instructions[:] = out
    return n_split


def _build_module() -> bass.Bass:
    nc = bass.Bass(
        "TRN2",
        target_bir_lowering=False,
        debug=False,
        enable_asserts=False,
        num_devices=N_CORES,
    )
    xb_d = nc.dram_tensor(
        "xb", [128, KS_BF, TOKENS], mybir.dt.bfloat16, kind="ExternalInput"
    ).ap()
    xf_d = nc.dram_tensor(
        "xf", [128, 2 * NPAIR, TOKENS], mybir.dt.float8e4, kind="ExternalInput"
    ).ap()
    wb_d = nc.dram_tensor(
        "wb", [128, NOB, KS_BF * 128], mybir.dt.float8e4, kind="ExternalInput"
    ).ap()
    wf_d = nc.dram_tensor(
        "wf", [128, NOB, 2 * NPAIR * 128], mybir.dt.float8e4,
        kind="ExternalInput"
    ).ap()
    out_d = nc.dram_tensor(
        "out", [OPAD, TOKENS], mybir.dt.float32, kind="ExternalOutput"
    ).ap()

    DR = mybir.MatmulPerfMode.DoubleRow
    with ExitStack() as ctx:
        tc = ctx.enter_context(tile.TileContext(nc))
        sb = ctx.enter_context(tc.tile_pool(name="sb", bufs=1))
        wbp = ctx.enter_context(tc.tile_pool(name="wbp", bufs=7))
        wfp = ctx.enter_context(tc.tile_pool(name="wfp", bufs=7))
        opool = ctx.enter_context(tc.tile_pool(name="opool", bufs=4))
        ps = ctx.enter_context(tc.tile_pool(name="ps", bufs=1, space="PSUM"))

        # Weight streams on the ACT HWDGE ring, o-block granular,
        # in consumption order.
        wb_sb = {}
        wf_sb = {}

        def _wf_load(ob):
            wf_sb[ob] = wfp.tile(
                [128, 2 * NPAIR, 128], mybir.dt.float8e4, name=f"wf{ob}",
                tag="wf"
            )
            nc.scalar.dma_start(out=wf_sb[ob], in_=wf_d[:, ob, :])

        def _wb_load(ob):
            wb_sb[ob] = wbp.tile(
                [128, KS_BF, 128], mybir.dt.float8e4, name=f"wb{ob}", tag="wb"
            )
            nc.scalar.dma_start(out=wb_sb[ob], in_=wb_d[:, ob, :])

        xb_sb = sb.tile([128, KS_BF, TOKENS], mybir.dt.bfloat16, name="xb_sb")
        xf_sb = sb.tile(
            [128, 2 * NPAIR, TOKENS], mybir.dt.float8e4, name="xf_sb"
        )
        # SP ring carries the whole x stream in consumption order (xf for
        # og0's leading fp8 phase, then xb in fine 2-subtile chunks so any
        # shortfall stalls in sub-us pieces that don't re-throttle HAM).
        # ACT ring carries weights: og0's first, then the rest.
        for lo, hi in [(0, 2), (2, 4), (4, 8), (8, 12), (12, 16)]:
            nc.sync.dma_start(
                out=xf_sb[:, lo:hi, :], in_=xf_d[:, lo:hi, :]
            )
        # og0's fp8 weights in two waves: the j0-j1 quarters (64 KB each)
        # land first so the ob-major opening stretch starts ~4 us earlier.
        for ob in range(4):
            wf_sb[ob] = wfp.tile(
                [128, 2 * NPAIR, 128], mybir.dt.float8e4, name=f"wf{ob}",
                tag="wf"
            )
            nc.scalar.dma_start(
                out=wf_sb[ob][:, 0:4, :], in_=wf_d[:, ob, 0:512]
            )
        for ob in range(4):
            nc.scalar.dma_start(
                out=wf_sb[ob][:, 4:2 * NPAIR, :],
                in_=wf_d[:, ob, 512:2 * NPAIR * 128],
            )
        for ob in range(4):
            _wb_load(ob)
        for c in range(0, KS_BF, 2):
            nc.sync.dma_start(
                out=xb_sb[:, c:c + 2, :], in_=xb_d[:, c:c + 2, :]
            )
        for ob in range(4, NOB):
            _wb_load(ob)
            _wf_load(ob)

        # PE prewarm: dummy matmuls on memset tiles while the first weight
        # tile is in flight, so real MMs start at HAM 8/8 (2.4 GHz).
        warm_a = sb.tile([128, 128], mybir.dt.bfloat16, name="warm_a")
        nc.gpsimd.memset(warm_a, 0.0)
        warm_b = sb.tile([128, 512], mybir.dt.bfloat16, name="warm_b")
        nc.gpsimd.memset(warm_b, 0.0)
        warm_ps = ps.tile([128, 512], mybir.dt.float32, name="warm_ps", tag="ps0")
        for i in range(3):
            nc.tensor.matmul(
                warm_ps, lhsT=warm_a, rhs=warm_b,
                start=(i == 0), stop=(i == 2),
            )

        pst = {}

        def bf_phase(obs, starting):
            for ks in range(KS_BF):
                for ob in obs:
                    lhsT = wb_sb[ob][:, ks]
                    st = starting and ks == 0
                    sp = (not starting) and ks == KS_BF - 1
                    nc.tensor.matmul(
                        pst[ob][0], lhsT=lhsT, rhs=xb_sb[:, ks, 0:512],
                        start=st, stop=sp,
                    )
                    nc.tensor.matmul(
                        pst[ob][1], lhsT=lhsT, rhs=xb_sb[:, ks, 512:1024],
                        start=st, stop=sp,
                    )

        def f8_phase(obs, starting):
            for j in range(NPAIR):
                for ob in obs:
                    lhsT = wf_sb[ob][:, 2 * j:2 * j + 2, :]
                    st = starting and j == 0
                    sp = (not starting) and j == NPAIR - 1
                    nc.tensor.matmul(
                        pst[ob][0], lhsT=lhsT,
                        rhs=xf_sb[:, 2 * j:2 * j + 2, 0:512],
                        start=st, stop=sp, perf_mode=DR,
                    )
                    nc.tensor.matmul(
                        pst[ob][1], lhsT=lhsT,
                        rhs=xf_sb[:, 2 * j:2 * j + 2, 512:1024],
                        start=st, stop=sp, perf_mode=DR,
                    )

        def alloc_ps(ob):
            bank = (ob % 4) * 2
            pst[ob] = (
                ps.tile([128, 512], mybir.dt.float32, name=f"p{ob}a",
                        tag=f"ps{bank}"),
                ps.tile([128, 512], mybir.dt.float32, name=f"p{ob}b",
                        tag=f"ps{bank + 1}"),
            )

        def evict(ob):
            ot = opool.tile([128, TOKENS], mybir.dt.float32, name="ot",
                            tag="ot")
            if ob % 2 == 0:
                nc.scalar.activation(
                    ot[:, 0:512], pst[ob][0],
                    mybir.ActivationFunctionType.Identity,
                )
                nc.vector.tensor_copy(out=ot[:, 512:1024], in_=pst[ob][1])
            else:
                nc.vector.tensor_copy(out=ot[:, 0:512], in_=pst[ob][0])
                nc.scalar.activation(
                    ot[:, 512:1024], pst[ob][1],
                    mybir.ActivationFunctionType.Identity,
                )
            eng = nc.sync if ob % 2 == 0 else nc.scalar
            eng.dma_start(
                out=out_d[ob * 128:(ob + 1) * 128, 0:512],
                in_=ot[:, 0:512],
            )
            eng.dma_start(
                out=out_d[ob * 128:(ob + 1) * 128, 512:1024],
                in_=ot[:, 512:1024],
            )

        # og0: 4 o-blocks, fp8 planes first — x_bf16 is still streaming in.
        # The leading stretch runs ob-major over pairs 0-1 so the very first
        # matmuls need only wf0 + xf chunk 0; later weights/chunks arrive
        # under its cover.
        og0 = [0, 1, 2, 3]
        for ob in og0:
            alloc_ps(ob)
        for ob in og0:
            for j in range(2):
                lhsT = wf_sb[ob][:, 2 * j:2 * j + 2, :]
                for th in range(2):
                    nc.tensor.matmul(
                        pst[ob][th], lhsT=lhsT,
                        rhs=xf_sb[:, 2 * j:2 * j + 2, th * 512:th * 512 + 512],
                        start=(j == 0), stop=False, perf_mode=DR,
                    )
        for j in range(2, NPAIR):
            for ob in og0:
                lhsT = wf_sb[ob][:, 2 * j:2 * j + 2, :]
                for th in range(2):
                    nc.tensor.matmul(
                        pst[ob][th], lhsT=lhsT,
                        rhs=xf_sb[:, 2 * j:2 * j + 2, th * 512:th * 512 + 512],
                        start=False, stop=False, perf_mode=DR,
                    )
        bf_phase(og0, False)
        for ob in og0:
            evict(ob)

        # Remaining o-blocks run serialized: each finishes (and its output
        # DMA issues) while the next block's matmuls run, so evictions and
        # stores spread across the span instead of bunching at the end.
        for ob in range(4, NOB - 1):
            alloc_ps(ob)
            bf_phase([ob], True)
            f8_phase([ob], False)
            evict(ob)

        # Final o-block: token-half-major — half 0's accumulation closes
        # first and its eviction + store run under half 1's matmuls, so the
        # tail only carries half 1's eviction and DMA.
        ob = NOB - 1
        alloc_ps(ob)
        ot = opool.tile([128, TOKENS], mybir.dt.float32, name="ot", tag="ot")
        for th in range(2):
            pt = pst[ob][th]
            rhs_lo = th * 512
            for ks in range(KS_BF):
                nc.tensor.matmul(
                    pt, lhsT=wb_sb[ob][:, ks],
                    rhs=xb_sb[:, ks, rhs_lo:rhs_lo + 512],
                    start=(ks == 0), stop=False,
                )
            for j in range(NPAIR):
                nc.tensor.matmul(
                    pt, lhsT=wf_sb[ob][:, 2 * j:2 * j + 2, :],
                    rhs=xf_sb[:, 2 * j:2 * j + 2, rhs_lo:rhs_lo + 512],
                    start=False, stop=(j == NPAIR - 1), perf_mode=DR,
                )
            for q in range(2):
                lo = th * 512 + q * 256
                src = pt[:, q * 256:q * 256 + 256]
                if q == 0:
                    nc.scalar.activation(
                        ot[:, lo:lo + 256], src,
                        mybir.ActivationFunctionType.Identity,
                    )
                else:
                    nc.vector.tensor_copy(out=ot[:, lo:lo + 256], in_=src)
                eng = nc.sync if q == 0 else nc.scalar
                eng.dma_start(
                    out=out_d[ob * 128:(ob + 1) * 128, lo:lo + 256],
                    in_=ot[:, lo:lo + 256],
                )
    _legalize_waits(nc)
    return nc


def _prep_inputs(x: np.ndarray, bp: np.ndarray):
    x = np.ascontiguousarray(x, dtype=np.float32)
    # xb[ki, ks, t] = x[t, ks*128 + ki]   (bf16)
    xb = np.ascontiguousarray(
        x[:, :KF0].T.reshape(KS_BF, 128, TOKENS).transpose(1, 0, 2)
    ).astype(BF16)
    # xf[ki, s, t] = x[t, KF0 + s*128 + ki]   (e4m3)
    xf = np.ascontiguousarray(
        x[:, KF0:].T.reshape(2 * NPAIR, 128, TOKENS).transpose(1, 0, 2)
    ).astype(FP8)

    # weights: bytes -> {0,1} bits (MSB-first) -> +-1
    bytes_m = bp.reshape(OUT_F, IN_F // 8).astype(np.uint8)
    w01 = np.unpackbits(bytes_m, axis=1)            # [OUT_F, IN_F]
    w_pm = (w01.astype(np.int8) << 1) - 1           # {-1, +1}

    in_maps = []
    for c in range(N_CORES):
        wc = np.zeros((OPAD, IN_F), dtype=np.int8)
        wc[:OS] = w_pm[c * OS:(c + 1) * OS]
        # wb[ki, ob, ks, oj] = w[ob*128+oj, ks*128+ki]
        wb = np.ascontiguousarray(
            wc[:, :KF0].T                     # [2048, 1408]
            .reshape(KS_BF, 128, NOB, 128)    # [ks, ki, ob, oj]
            .transpose(1, 2, 0, 3)            # [ki, ob, ks, oj]
            .reshape(128, NOB, KS_BF * 128)
        ).astype(FP8)
        wf = np.ascontiguousarray(
            wc[:, KF0:].T
            .reshape(2 * NPAIR, 128, NOB, 128)
            .transpose(1, 2, 0, 3)
            .reshape(128, NOB, 2 * NPAIR * 128)
        ).astype(FP8)
        in_maps.append({"xb": xb, "xf": xf, "wb": wb, "wf": wf})
    return in_maps


def _run(x: np.ndarray, bp: np.ndarray, **spmd_kwargs):
    if "nc" not in _CACHE:
        _CACHE["nc"] = _build_module()
    nc = _CACHE["nc"]
    in_maps = _prep_inputs(x, bp)
    res = run_bass_kernel_spmd(
        nc, in_maps, core_ids=list(range(N_CORES)), **spmd_kwargs
    )
    out = np.ascontiguousarray(
        np.concatenate([r["out"][:OS] for r in res.results], axis=0).T
    )
    return out, res


def _host_reference(x: np.ndarray, bp: np.ndarray) -> np.ndarray:
    # Safety net for inputs outside the fast path's envelope.
    shifts = np.arange(7, -1, -1)
    bits = (bp.astype(np.int64)[:, None] >> shifts) & 1
    w = bits.reshape(OUT_F, IN_F).astype(np.float32) * 2 - 1
    return (x @ w.T).astype(np.float32)


def kernel(x: np.ndarray, bp: np.ndarray) -> np.ndarray:
    x = np.asarray(x, dtype=np.float32)
    bp = np.asarray(bp)
    # e4m3 covers |x| < 224 exactly in range; standard-normal inputs sit
    # near 5.2. Anything wilder goes through the host fallback.
    if (not np.isfinite(x).all()) or np.abs(x).max() >= 224.0 \
            or bp.min() < 0 or bp.max() > 255:
        return _host_reference(x, bp)
    out, _ = _run(x, bp)
    return out
